# revision 13
# baseline (speedup 1.0000x reference)
"""Paged GQA chunked-prefill attention for 8 Trainium2 NeuronCores.

Problem (hardcoded): B=4 seqs x Q=256 new tokens, H=32 query heads, KVH=8 kv
heads (GQA group G=4), D=128 head dim, paged KV cache of 512 blocks x 16
tokens, per-seq lengths in seq_lens (clamped to >= Q), causal masking.

Sharding: tensor-parallel over heads. Core h gets kv head h and query heads
h*4..h*4+3; block_tables/seq_lens are resolved host-side while packing the
shards; the output is all-gathered host-side over the hidden dim.

Per-core device kernel (seq b, kv chunk c of 128 positions, q = (t,g) -> 1024
columns, two 512-column halves n):
  S^T[kv,qh] = K_c^T q            (bf16 matmul pair sharing one LDWEIGHTS)
  S^T += causal mask              (identity-lhsT matmul, boundary chunks only)
  U = exp(SCALE * S^T)            (ScalarE, one 1024-col activation, bf16 out)
  O^T[d,qh] += V_c^T @ U          (PSUM accumulation, O pair shares LDWEIGHTS)
  denominator: U tiles are binomial-merged on the idle DVE (bf16 adds), so
  the PE runs ONE ones^T matmul per (seq, half) instead of one per chunk.
Per-seq: copy O^T (bf16, GpSimd; DVE for the last seq) and l (f32, ScalarE)
to SBUF, DMA out. The softmax division O/l happens on the HOST during
unpacking -- no device reciprocal/broadcast/multiply epilogue.

PE emission is software-pipelined: S(c+1) is issued before the O matmuls of
chunk c so the tensor engine always has independent work while exp(c) runs.
"""
import math

import ml_dtypes
import numpy as np

import concourse.mybir as mybir
import concourse.tile as tile
from concourse import bacc
from concourse.bass_utils import run_bass_kernel_spmd

B, Q, H, D = 4, 256, 32, 128
KVH = 8
G = H // KVH
BLOCK = 16
NB = 128
KV = NB * BLOCK
NUM_BLOCKS = B * NB
SCALE = 1.0 / math.sqrt(D)
N_CORES = 8
CHUNK = 128
QCOLS = G * Q  # 1024 q columns per sequence per core
NHALF = 512

F32 = mybir.dt.float32
BF16 = mybir.dt.bfloat16
NEG = -1.0e9


def _plan(seq_lens):
    """Per-seq chunk counts, offsets, and boundary-chunk mask tiles."""
    L = np.maximum(np.asarray(seq_lens, dtype=np.int64), Q)
    cb = [int((int(Lb) + CHUNK - 1) // CHUNK) for Lb in L]
    offs = np.concatenate([[0], np.cumsum(cb)]).astype(int)
    masked = []  # list of (b, c, mask[128,256])
    t = np.arange(Q)
    p = np.arange(CHUNK)
    for b in range(B):
        Lb = int(L[b])
        for c in range(cb[b]):
            if c * CHUNK + CHUNK - 1 > Lb - Q:
                kvpos = c * CHUNK + p
                m = np.where(
                    kvpos[:, None] > (Lb - Q) + t[None, :], NEG, 0.0
                ).astype(np.float32)
                masked.append((b, c, m))
    return L, cb, offs, masked


def _half_state(L, b, c, n):
    # 'skip' = every q in the half is masked for this chunk;
    # 'mask' = the causal diagonal crosses this (chunk, half)
    lo = int(L[b]) - Q + n * CHUNK
    if c * CHUNK > lo + CHUNK - 1:
        return "skip"
    if c * CHUNK + CHUNK - 1 > lo:
        return "mask"
    return "clear"


def _build(seq_lens):
    L, cb, offs, masked = _plan(seq_lens)
    C = int(offs[-1])
    nmask = len(masked)
    border = sorted(range(B), key=lambda b: cb[b])  # shortest first
    # order mask tiles by processing order so the early ones land first
    order = sorted(
        range(len(masked)),
        key=lambda i: (border.index(masked[i][0]), masked[i][1]),
    )
    masked = [masked[i] for i in order]
    mask_np = np.concatenate([m for _, _, m in masked], axis=1).astype(
        ml_dtypes.bfloat16
    )  # [128, nm*256]; 0/-1e9 are bf16-exact
    mask_idx = {(b, c): i for i, (b, c, _) in enumerate(masked)}
    identb_np = np.eye(CHUNK, dtype=ml_dtypes.bfloat16)
    ones_np = np.ones((CHUNK, 2), dtype=ml_dtypes.bfloat16)

    nc = bacc.Bacc(
        "TRN2", target_bir_lowering=False, debug=False, num_devices=N_CORES
    )
    kt_d = nc.dram_tensor("kt", [D, C * CHUNK], BF16, kind="ExternalInput")
    v_d = nc.dram_tensor("v", [CHUNK, C * CHUNK], BF16, kind="ExternalInput")
    qt_d = nc.dram_tensor("qt", [D, B * QCOLS], BF16, kind="ExternalInput")
    oo_d = nc.dram_tensor("out_o", [B, D, QCOLS], BF16, kind="ExternalOutput")
    ol_d = nc.dram_tensor("out_l", [2, B * QCOLS], F32, kind="ExternalOutput")
    mask_d = nc.inline_tensor(mask_np, name="mask_const")
    identb_d = nc.inline_tensor(identb_np, name="identb_const")
    ones_d = nc.inline_tensor(ones_np, name="ones_const")

    exp = mybir.ActivationFunctionType.Exp

    with tile.TileContext(nc) as tc:
        with (
            tc.tile_pool(name="sbin", bufs=1) as sbin,
            tc.tile_pool(name="sbu", bufs=4) as sbu,
            tc.tile_pool(name="sbt", bufs=10) as sbt,
            tc.tile_pool(name="sbe", bufs=2) as sbe,
            tc.tile_pool(name="ps_s", bufs=2, space="PSUM") as ps_s,
            tc.tile_pool(name="ps_o", bufs=1, space="PSUM") as ps_o,
            tc.tile_pool(name="ps_l", bufs=1, space="PSUM") as ps_l,
        ):
            kt_t = [None] * B
            qt_t = [None] * B
            v_t = [None] * B
            for b in border:
                w = cb[b] * CHUNK
                kt_t[b] = sbin.tile([D, w], BF16, tag=f"kt{b}", name=f"kt{b}")
                qt_t[b] = sbin.tile(
                    [D, QCOLS], BF16, tag=f"qt{b}", name=f"qt{b}"
                )
                v_t[b] = sbin.tile([CHUNK, w], BF16, tag=f"v{b}", name=f"v{b}")
            identr = sbin.tile([CHUNK, CHUNK], BF16, tag="identr")
            ones = sbin.tile([CHUNK, 2], BF16, tag="ones")
            masks = sbin.tile([CHUNK, nmask * Q], BF16, tag="masks")
            lall = sbe.tile([2, B * QCOLS], F32, tag="lall")

            # DMA initiators are sync/scalar/gpsimd only. The first compute
            # needs b0's K chunk 0 + first q half: issue those two first on
            # separate queues, then stream the rest on sync in processing
            # order. ScalarE issues one DMA then is free for exps.
            b0 = border[0]
            w0 = cb[b0] * CHUNK
            oo0 = offs[b0] * CHUNK
            nc.sync.dma_start(
                kt_t[b0][:, 0:CHUNK], kt_d.ap()[:, oo0 : oo0 + CHUNK]
            )
            nc.scalar.dma_start(
                qt_t[b0][:, 0:NHALF],
                qt_d.ap()[:, b0 * QCOLS : b0 * QCOLS + NHALF],
            )
            nc.sync.dma_start(
                qt_t[b0][:, NHALF:QCOLS],
                qt_d.ap()[:, b0 * QCOLS + NHALF : (b0 + 1) * QCOLS],
            )
            if w0 > CHUNK:
                nc.sync.dma_start(
                    kt_t[b0][:, CHUNK:w0], kt_d.ap()[:, oo0 + CHUNK : oo0 + w0]
                )
            nc.sync.dma_start(v_t[b0][:], v_d.ap()[:, oo0 : oo0 + w0])
            # identity+masks are needed during the very first chunk (the
            # shortest seq is boundary-heavy), ones at the first seq end.
            nc.gpsimd.dma_start(identr[:], identb_d.ap())
            cut = Q * sum(1 for bb, _, _ in masked if cb[bb] <= cb[border[1]])
            cut = max(Q, min(cut, nmask * Q))
            nc.gpsimd.dma_start(masks[:, 0:cut], mask_d.ap()[:, 0:cut])
            nc.gpsimd.dma_start(ones[:], ones_d.ap())
            # the second seq's K/Q ride the otherwise-idle scalar queue so
            # they don't sit behind b0's bulk on sync (seq-transition stall)
            for bi, b in enumerate(border[1:]):
                w = cb[b] * CHUNK
                o0 = offs[b] * CHUNK
                keng = nc.scalar if bi == 0 else nc.sync
                keng.dma_start(kt_t[b][:], kt_d.ap()[:, o0 : o0 + w])
                keng.dma_start(
                    qt_t[b][:], qt_d.ap()[:, b * QCOLS : (b + 1) * QCOLS]
                )
                nc.sync.dma_start(v_t[b][:], v_d.ap()[:, o0 : o0 + w])
            if cut < nmask * Q:
                nc.gpsimd.dma_start(
                    masks[:, cut : nmask * Q], mask_d.ap()[:, cut : nmask * Q]
                )

            # ---- compute ------------------------------------------------
            sched = [(b, c) for b in border for c in range(cb[b])]

            def emit_score(b, c):
                """S matmul pair + mask adds + one exp; returns (u, states)."""
                states = [_half_state(L, b, c, n) for n in range(2)]
                s_ps = ps_s.tile([CHUNK, QCOLS], F32, tag="s")
                for n in range(2):
                    if states[n] == "skip":
                        continue
                    half = slice(n * NHALF, (n + 1) * NHALF)
                    nc.tensor.matmul(
                        s_ps[:, half],
                        kt_t[b][:, c * CHUNK : (c + 1) * CHUNK],
                        qt_t[b][:, half],
                        start=True,
                        stop=states[n] == "clear",
                    )
                for n in range(2):
                    if states[n] == "mask":
                        mi = mask_idx[(b, c)]
                        half = slice(n * NHALF, (n + 1) * NHALF)
                        mb = (
                            masks[
                                :,
                                mi * Q + n * CHUNK : mi * Q + (n + 1) * CHUNK,
                            ]
                            .unsqueeze(2)
                            .broadcast_to([CHUNK, CHUNK, G])
                        )
                        nc.tensor.matmul(
                            s_ps[:, half], identr[:], mb, start=False, stop=True
                        )
                lo = 0 if states[0] != "skip" else NHALF
                hi = QCOLS if states[1] != "skip" else NHALF
                u = sbu.tile([CHUNK, QCOLS], BF16, tag="u")
                nc.scalar.activation(
                    u[:, lo:hi], s_ps[:, lo:hi], exp, scale=SCALE
                )
                return u, states

            # binomial merge stacks per (seq, half): list of (level, ap)
            stacks = {}
            tcount = [0]

            def push_merge(key, ap):
                st = stacks.setdefault(key, [])
                st.append((0, ap))
                while len(st) >= 2 and st[-1][0] == st[-2][0]:
                    lv, a1 = st.pop()
                    _, a0 = st.pop()
                    tcount[0] += 1
                    s = sbt.tile(
                        [CHUNK, NHALF],
                        BF16,
                        tag="ts",
                        name=f"ts{tcount[0]}",
                    )
                    nc.vector.tensor_add(s[:], a0, a1)
                    st.append((lv + 1, s[:]))

            def flush_stack(key):
                st = stacks.get(key, [])
                while len(st) >= 2:
                    _, a1 = st.pop()
                    lv, a0 = st.pop()
                    tcount[0] += 1
                    s = sbt.tile(
                        [CHUNK, NHALF],
                        BF16,
                        tag="ts",
                        name=f"ts{tcount[0]}",
                    )
                    nc.vector.tensor_add(s[:], a0, a1)
                    st.append((lv + 1, s[:]))
                return st[0][1] if st else None

            pending = None
            seq_state = {}
            terminal_b = border[-1]
            for i, (b, c) in enumerate(sched):
                if c == 0:
                    seq_state[b] = (
                        ps_o.tile([D, QCOLS], F32, tag="o", name="o"),
                        [
                            min(
                                cb[b] - 1,
                                (int(L[b]) - Q + n * CHUNK + CHUNK - 1)
                                // CHUNK,
                            )
                            for n in range(2)
                        ],
                        [
                            ps_l.tile([2, NHALF], F32, tag="l0", name="l0"),
                            ps_l.tile([2, NHALF], F32, tag="l1", name="l1"),
                        ],
                    )
                if pending is None:
                    pending = emit_score(b, c)
                u, states = pending
                pending = emit_score(*sched[i + 1]) if i + 1 < len(sched) else None
                o_ps, last_n, l_ps = seq_state[b]
                # O pair first (shared V weights), then DVE merge pushes
                for n in range(2):
                    if states[n] == "skip":
                        continue
                    half = slice(n * NHALF, (n + 1) * NHALF)
                    nc.tensor.matmul(
                        o_ps[:, half],
                        v_t[b][:, c * CHUNK : (c + 1) * CHUNK],
                        u[:, half],
                        start=c == 0,
                        stop=c == last_n[n],
                    )
                for n in range(2):
                    if states[n] == "skip":
                        continue
                    half = slice(n * NHALF, (n + 1) * NHALF)
                    if b != terminal_b:
                        push_merge((b, n), u[:, half])
                        continue
                    # terminal seq: tree the body, feed the last two chunks
                    # straight to the PE so the tail has no deep DVE flush
                    tail_lo = max(0, last_n[n] - 1)
                    if c < tail_lo:
                        push_merge((b, n), u[:, half])
                    elif c == tail_lo:
                        root = flush_stack((b, n))
                        if root is not None:
                            nc.tensor.matmul(
                                l_ps[n][:], ones[:, 0:2], root,
                                start=True, stop=False,
                            )
                        nc.tensor.matmul(
                            l_ps[n][:], ones[:, 0:2], u[:, half],
                            start=root is None, stop=c == last_n[n],
                        )
                    elif c == last_n[n]:
                        nc.tensor.matmul(
                            l_ps[n][:], ones[:, 0:2], u[:, half],
                            start=False, stop=True,
                        )

                if c == cb[b] - 1:
                    terminal = b == terminal_b
                    if not terminal:
                        for n in range(2):
                            root = flush_stack((b, n))
                            nc.tensor.matmul(
                                l_ps[n][:], ones[:, 0:2], root,
                                start=True, stop=True,
                            )
                    for n in range(2):
                        nc.vector.tensor_copy(
                            lall[:, b * QCOLS + n * NHALF :
                                 b * QCOLS + (n + 1) * NHALF],
                            l_ps[n][:],
                        )
                    o_sb = sbe.tile([D, QCOLS], BF16, tag="osb")
                    if terminal:
                        # ScalarE is idle after the last exp; overlaps the
                        # DVE l copies
                        nc.scalar.copy(o_sb[:], o_ps[:])
                    else:
                        nc.vector.tensor_copy(o_sb[:], o_ps[:])
                    nc.sync.dma_start(oo_d.ap()[b], o_sb[:])
                    if terminal:
                        nc.gpsimd.dma_start(ol_d.ap(), lall[:])

    nc.compile()
    return nc, L, cb, offs


def _pack_inputs(query, k_cache, v_cache, block_tables, L, cb, offs):
    """Gather the paged cache and pack per-core shards in device layouts."""
    C = int(offs[-1])
    k_lin = k_cache[block_tables].reshape(B, KV, KVH, D)
    v_lin = v_cache[block_tables].reshape(B, KV, KVH, D)
    kt_all = np.zeros((KVH, D, C * CHUNK), dtype=np.float32)
    v_all = np.zeros((KVH, CHUNK, C * CHUNK), dtype=np.float32)
    for b in range(B):
        Lb, w = int(L[b]), cb[b] * CHUNK
        kk = np.zeros((w, KVH, D), dtype=np.float32)
        kk[:Lb] = k_lin[b, :Lb]
        # [w, KVH, D] -> [KVH, D, w]
        kt_all[:, :, offs[b] * CHUNK : offs[b] * CHUNK + w] = kk.transpose(
            1, 2, 0
        )
        vv = np.zeros((w, KVH, D), dtype=np.float32)
        vv[:Lb] = v_lin[b, :Lb]
        # [cb, 128, KVH, D] -> [KVH, 128, cb, D] -> [KVH, 128, w]
        v_all[:, :, offs[b] * CHUNK : offs[b] * CHUNK + w] = (
            vv.reshape(cb[b], CHUNK, KVH, D)
            .transpose(2, 1, 0, 3)
            .reshape(KVH, CHUNK, w)
        )
    # query [B,Q,H,D] -> [KVH, D, B, Q, G] (t-major, g inner)
    qt_all = (
        query.transpose(2, 3, 0, 1)
        .reshape(KVH, G, D, B, Q)
        .transpose(0, 2, 3, 4, 1)
        .reshape(KVH, D, B * QCOLS)
    )
    qt_all = np.ascontiguousarray(qt_all, dtype=np.float32)
    return [
        {
            "kt": np.ascontiguousarray(kt_all[h]).astype(ml_dtypes.bfloat16),
            "v": np.ascontiguousarray(v_all[h]).astype(ml_dtypes.bfloat16),
            "qt": qt_all[h].astype(ml_dtypes.bfloat16),
        }
        for h in range(KVH)
    ]


def _unpack_outputs(results):
    """Host softmax division + relayout.

    Per core: out_o [B,D,QCOLS] bf16 (unnormalized O^T, q=(t,g) cols) and
    out_l [2,B*QCOLS] f32 where row 0 holds the denominators.
    """
    out = np.empty((B * Q, H * D), dtype=np.float32)
    for h, res in enumerate(results):
        o = np.asarray(res["out_o"], dtype=np.float32)  # [B, D, QCOLS]
        l = np.asarray(res["out_l"], dtype=np.float32)[0].reshape(B, QCOLS)
        o = o / l[:, None, :]
        o = o.reshape(B, D, Q, G).transpose(0, 2, 3, 1).reshape(B * Q, G * D)
        out[:, h * G * D : (h + 1) * G * D] = o
    return out


def kernel(query, k_cache, v_cache, block_tables, seq_lens):
    query = np.asarray(query, dtype=np.float32)
    k_cache = np.asarray(k_cache, dtype=np.float32)
    v_cache = np.asarray(v_cache, dtype=np.float32)
    block_tables = np.asarray(block_tables, dtype=np.int64)
    nc, L, cb, offs = _build(np.asarray(seq_lens))
    in_maps = _pack_inputs(query, k_cache, v_cache, block_tables, L, cb, offs)
    res = run_bass_kernel_spmd(nc, in_maps, core_ids=list(range(N_CORES)))
    return _unpack_outputs(res.results)


# revision 16
# speedup vs baseline: 1.1384x; 1.1384x over previous
"""Paged GQA chunked-prefill attention for 8 Trainium2 NeuronCores.

Problem (hardcoded): B=4 seqs x Q=256 new tokens, H=32 query heads, KVH=8 kv
heads (GQA group G=4), D=128 head dim, paged KV cache of 512 blocks x 16
tokens, per-seq lengths in seq_lens (clamped to >= Q), causal masking.

Sharding: tensor-parallel over heads. Core h gets kv head h and query heads
h*4..h*4+3; block_tables/seq_lens are resolved host-side while packing the
shards; the output is all-gathered host-side over the hidden dim.

Per-core device kernel (seq b, kv chunk c of 128 positions, q = (t,g) -> 1024
columns, two 512-column halves n):
  S^T[kv,qh] = K_c^T q            (bf16 matmul pair sharing one LDWEIGHTS)
  S^T += causal mask              (identity-lhsT matmul, boundary chunks only)
  U = exp(SCALE * S^T)            (ScalarE, one 1024-col activation, bf16 out)
  O^T[d,qh] += V_c^T @ U          (PSUM accumulation, O pair shares LDWEIGHTS)
  denominator: U tiles are binomial-merged on the idle DVE (bf16 adds), so
  the PE runs ONE ones^T matmul per (seq, half) instead of one per chunk.
Per-seq: copy O^T (bf16, GpSimd; DVE for the last seq) and l (f32, ScalarE)
to SBUF, DMA out. The softmax division O/l happens on the HOST during
unpacking -- no device reciprocal/broadcast/multiply epilogue.

PE emission is software-pipelined: S(c+1) is issued before the O matmuls of
chunk c so the tensor engine always has independent work while exp(c) runs.
"""
import math

import ml_dtypes
import numpy as np

import concourse.mybir as mybir
import concourse.tile as tile
from concourse import bacc
from concourse.bass_utils import run_bass_kernel_spmd

B, Q, H, D = 4, 256, 32, 128
KVH = 8
G = H // KVH
BLOCK = 16
NB = 128
KV = NB * BLOCK
NUM_BLOCKS = B * NB
SCALE = 1.0 / math.sqrt(D)
N_CORES = 8
CHUNK = 128
QCOLS = G * Q  # 1024 q columns per sequence per core
NHALF = 512

F32 = mybir.dt.float32
BF16 = mybir.dt.bfloat16
NEG = -1.0e9


def _plan(seq_lens):
    """Per-seq chunk counts, offsets, and boundary-chunk mask tiles."""
    L = np.maximum(np.asarray(seq_lens, dtype=np.int64), Q)
    cb = [int((int(Lb) + CHUNK - 1) // CHUNK) for Lb in L]
    offs = np.concatenate([[0], np.cumsum(cb)]).astype(int)
    masked = []  # list of (b, c, mask[128,256])
    t = np.arange(Q)
    p = np.arange(CHUNK)
    for b in range(B):
        Lb = int(L[b])
        for c in range(cb[b]):
            if c * CHUNK + CHUNK - 1 > Lb - Q:
                kvpos = c * CHUNK + p
                m = np.where(
                    kvpos[:, None] > (Lb - Q) + t[None, :], NEG, 0.0
                ).astype(np.float32)
                masked.append((b, c, m))
    return L, cb, offs, masked


def _half_state(L, b, c, n):
    # 'skip' = every q in the half is masked for this chunk;
    # 'mask' = the causal diagonal crosses this (chunk, half)
    lo = int(L[b]) - Q + n * CHUNK
    if c * CHUNK > lo + CHUNK - 1:
        return "skip"
    if c * CHUNK + CHUNK - 1 > lo:
        return "mask"
    return "clear"


def _build(seq_lens):
    L, cb, offs, masked = _plan(seq_lens)
    C = int(offs[-1])
    nmask = len(masked)
    border = sorted(range(B), key=lambda b: cb[b])  # shortest first
    # order mask tiles by processing order so the early ones land first
    order = sorted(
        range(len(masked)),
        key=lambda i: (border.index(masked[i][0]), masked[i][1]),
    )
    masked = [masked[i] for i in order]
    mask_np = np.concatenate([m for _, _, m in masked], axis=1).astype(
        ml_dtypes.bfloat16
    )  # [128, nm*256]; 0/-1e9 are bf16-exact
    mask_idx = {(b, c): i for i, (b, c, _) in enumerate(masked)}
    identb_np = np.eye(CHUNK, dtype=ml_dtypes.bfloat16)
    ones_np = np.ones((CHUNK, 2), dtype=ml_dtypes.bfloat16)

    nc = bacc.Bacc(
        "TRN2", target_bir_lowering=False, debug=False, num_devices=N_CORES
    )
    kt_d = nc.dram_tensor("kt", [D, C * CHUNK], BF16, kind="ExternalInput")
    v_d = nc.dram_tensor("v", [CHUNK, C * CHUNK], BF16, kind="ExternalInput")
    qt_d = nc.dram_tensor("qt", [D, B * QCOLS], BF16, kind="ExternalInput")
    oo_d = nc.dram_tensor("out_o", [B, D, QCOLS], BF16, kind="ExternalOutput")
    ol_d = nc.dram_tensor("out_l", [2, B * QCOLS], F32, kind="ExternalOutput")
    mask_d = nc.inline_tensor(mask_np, name="mask_const")
    identb_d = nc.inline_tensor(identb_np, name="identb_const")
    ones_d = nc.inline_tensor(ones_np, name="ones_const")

    exp = mybir.ActivationFunctionType.Exp

    with tile.TileContext(nc) as tc:
        with (
            tc.tile_pool(name="sbin", bufs=1) as sbin,
            tc.tile_pool(name="sbu", bufs=4) as sbu,
            tc.tile_pool(name="sbt", bufs=10) as sbt,
            tc.tile_pool(name="sbe", bufs=2) as sbe,
            tc.tile_pool(name="ps_s", bufs=2, space="PSUM") as ps_s,
            tc.tile_pool(name="ps_o", bufs=1, space="PSUM") as ps_o,
            tc.tile_pool(name="ps_l", bufs=1, space="PSUM") as ps_l,
        ):
            kt_t = [None] * B
            qt_t = [None] * B
            v_t = [None] * B
            for b in border:
                w = cb[b] * CHUNK
                kt_t[b] = sbin.tile([D, w], BF16, tag=f"kt{b}", name=f"kt{b}")
                qt_t[b] = sbin.tile(
                    [D, QCOLS], BF16, tag=f"qt{b}", name=f"qt{b}"
                )
                v_t[b] = sbin.tile([CHUNK, w], BF16, tag=f"v{b}", name=f"v{b}")
            identr = sbin.tile([CHUNK, CHUNK], BF16, tag="identr")
            ones = sbin.tile([CHUNK, 2], BF16, tag="ones")
            masks = sbin.tile([CHUNK, nmask * Q], BF16, tag="masks")
            lall = sbe.tile([2, B * QCOLS], F32, tag="lall")

            # DMA initiators are sync/scalar/gpsimd only. The first compute
            # needs b0's K chunk 0 + first q half: issue those two first on
            # separate queues, then stream the rest on sync in processing
            # order. ScalarE issues one DMA then is free for exps.
            b0 = border[0]
            w0 = cb[b0] * CHUNK
            oo0 = offs[b0] * CHUNK
            nc.sync.dma_start(
                kt_t[b0][:, 0:CHUNK], kt_d.ap()[:, oo0 : oo0 + CHUNK]
            )
            nc.scalar.dma_start(
                qt_t[b0][:, 0:NHALF],
                qt_d.ap()[:, b0 * QCOLS : b0 * QCOLS + NHALF],
            )
            nc.sync.dma_start(
                qt_t[b0][:, NHALF:QCOLS],
                qt_d.ap()[:, b0 * QCOLS + NHALF : (b0 + 1) * QCOLS],
            )
            if w0 > CHUNK:
                nc.sync.dma_start(
                    kt_t[b0][:, CHUNK:w0], kt_d.ap()[:, oo0 + CHUNK : oo0 + w0]
                )
            # identity+masks are needed during the very first chunk (the
            # shortest seq is boundary-heavy), ones at the first seq end.
            # consts on gpsimd: identity first, then only b0's mask tiles so
            # the first boundary chunk isn't gated on the full mask blob
            nc.gpsimd.dma_start(identr[:], identb_d.ap())
            cut0 = Q * sum(1 for bb, _, _ in masked if bb == border[0])
            cut0 = max(Q, min(cut0, nmask * Q))
            nc.gpsimd.dma_start(masks[:, 0:cut0], mask_d.ap()[:, 0:cut0])
            nc.gpsimd.dma_start(ones[:], ones_d.ap())
            cut = Q * sum(1 for bb, _, _ in masked if cb[bb] <= cb[border[1]])
            cut = max(cut0, min(cut, nmask * Q))
            if cut > cut0:
                nc.gpsimd.dma_start(
                    masks[:, cut0:cut], mask_d.ap()[:, cut0:cut]
                )
            # sync queue: pull each later seq's K ahead of the previous V so
            # sequence transitions aren't DMA-gated
            prev_v = [(v_t[b0], oo0, w0)]
            for b in border[1:]:
                w = cb[b] * CHUNK
                o0 = offs[b] * CHUNK
                nc.sync.dma_start(kt_t[b][:], kt_d.ap()[:, o0 : o0 + w])
                if prev_v:
                    vt, vo, vw = prev_v.pop()
                    nc.sync.dma_start(vt[:], v_d.ap()[:, vo : vo + vw])
                nc.sync.dma_start(
                    qt_t[b][:], qt_d.ap()[:, b * QCOLS : (b + 1) * QCOLS]
                )
                prev_v.append((v_t[b], o0, w))
            if prev_v:
                vt, vo, vw = prev_v.pop()
                nc.sync.dma_start(vt[:], v_d.ap()[:, vo : vo + vw])
            if cut < nmask * Q:
                nc.gpsimd.dma_start(
                    masks[:, cut : nmask * Q], mask_d.ap()[:, cut : nmask * Q]
                )

            # ---- compute ------------------------------------------------
            sched = [(b, c) for b in border for c in range(cb[b])]

            def emit_score(b, c):
                """S matmul pair + mask adds + one exp; returns (u, states)."""
                states = [_half_state(L, b, c, n) for n in range(2)]
                s_ps = ps_s.tile([CHUNK, QCOLS], F32, tag="s")
                for n in range(2):
                    if states[n] == "skip":
                        continue
                    half = slice(n * NHALF, (n + 1) * NHALF)
                    nc.tensor.matmul(
                        s_ps[:, half],
                        kt_t[b][:, c * CHUNK : (c + 1) * CHUNK],
                        qt_t[b][:, half],
                        start=True,
                        stop=states[n] == "clear",
                    )
                for n in range(2):
                    if states[n] == "mask":
                        mi = mask_idx[(b, c)]
                        half = slice(n * NHALF, (n + 1) * NHALF)
                        mb = (
                            masks[
                                :,
                                mi * Q + n * CHUNK : mi * Q + (n + 1) * CHUNK,
                            ]
                            .unsqueeze(2)
                            .broadcast_to([CHUNK, CHUNK, G])
                        )
                        nc.tensor.matmul(
                            s_ps[:, half], identr[:], mb, start=False, stop=True
                        )
                lo = 0 if states[0] != "skip" else NHALF
                hi = QCOLS if states[1] != "skip" else NHALF
                u = sbu.tile([CHUNK, QCOLS], BF16, tag="u")
                nc.scalar.activation(
                    u[:, lo:hi], s_ps[:, lo:hi], exp, scale=SCALE
                )
                return u, states

            # binomial merge stacks per (seq, half): list of (level, ap)
            stacks = {}
            tcount = [0]

            def push_merge(key, ap):
                st = stacks.setdefault(key, [])
                st.append((0, ap))
                while len(st) >= 2 and st[-1][0] == st[-2][0]:
                    lv, a1 = st.pop()
                    _, a0 = st.pop()
                    tcount[0] += 1
                    s = sbt.tile(
                        [CHUNK, NHALF],
                        BF16,
                        tag="ts",
                        name=f"ts{tcount[0]}",
                    )
                    nc.vector.tensor_add(s[:], a0, a1)
                    st.append((lv + 1, s[:]))

            def flush_stack(key):
                st = stacks.get(key, [])
                while len(st) >= 2:
                    _, a1 = st.pop()
                    lv, a0 = st.pop()
                    tcount[0] += 1
                    s = sbt.tile(
                        [CHUNK, NHALF],
                        BF16,
                        tag="ts",
                        name=f"ts{tcount[0]}",
                    )
                    nc.vector.tensor_add(s[:], a0, a1)
                    st.append((lv + 1, s[:]))
                return st[0][1] if st else None

            pending = None
            seq_state = {}
            terminal_b = border[-1]
            for i, (b, c) in enumerate(sched):
                if c == 0:
                    seq_state[b] = (
                        ps_o.tile([D, QCOLS], F32, tag="o", name="o"),
                        [
                            min(
                                cb[b] - 1,
                                (int(L[b]) - Q + n * CHUNK + CHUNK - 1)
                                // CHUNK,
                            )
                            for n in range(2)
                        ],
                        [
                            ps_l.tile([2, NHALF], F32, tag="l0", name="l0"),
                            ps_l.tile([2, NHALF], F32, tag="l1", name="l1"),
                        ],
                    )
                if pending is None:
                    pending = emit_score(b, c)
                u, states = pending
                pending = emit_score(*sched[i + 1]) if i + 1 < len(sched) else None
                o_ps, last_n, l_ps = seq_state[b]
                # O pair first (shared V weights), then DVE merge pushes
                for n in range(2):
                    if states[n] == "skip":
                        continue
                    half = slice(n * NHALF, (n + 1) * NHALF)
                    nc.tensor.matmul(
                        o_ps[:, half],
                        v_t[b][:, c * CHUNK : (c + 1) * CHUNK],
                        u[:, half],
                        start=c == 0,
                        stop=c == last_n[n],
                    )
                for n in range(2):
                    if states[n] == "skip":
                        continue
                    half = slice(n * NHALF, (n + 1) * NHALF)
                    push_merge((b, n), u[:, half])

                if c == cb[b] - 1:
                    terminal = b == terminal_b
                    for n in range(2):
                        root = flush_stack((b, n))
                        nc.tensor.matmul(
                            l_ps[n][:], ones[:, 0:2], root,
                            start=True, stop=True,
                        )
                    for n in range(2):
                        nc.vector.tensor_copy(
                            lall[:, b * QCOLS + n * NHALF :
                                 b * QCOLS + (n + 1) * NHALF],
                            l_ps[n][:],
                        )
                    o_sb = sbe.tile([D, QCOLS], BF16, tag="osb")
                    if terminal:
                        # ScalarE is idle after the last exp; overlaps the
                        # DVE l copies
                        nc.scalar.copy(o_sb[:], o_ps[:])
                    else:
                        nc.vector.tensor_copy(o_sb[:], o_ps[:])
                    nc.sync.dma_start(oo_d.ap()[b], o_sb[:])
                    if terminal:
                        nc.gpsimd.dma_start(ol_d.ap(), lall[:])

    nc.compile()
    return nc, L, cb, offs


def _pack_inputs(query, k_cache, v_cache, block_tables, L, cb, offs):
    """Gather the paged cache and pack per-core shards in device layouts."""
    C = int(offs[-1])
    k_lin = k_cache[block_tables].reshape(B, KV, KVH, D)
    v_lin = v_cache[block_tables].reshape(B, KV, KVH, D)
    kt_all = np.zeros((KVH, D, C * CHUNK), dtype=np.float32)
    v_all = np.zeros((KVH, CHUNK, C * CHUNK), dtype=np.float32)
    for b in range(B):
        Lb, w = int(L[b]), cb[b] * CHUNK
        kk = np.zeros((w, KVH, D), dtype=np.float32)
        kk[:Lb] = k_lin[b, :Lb]
        # [w, KVH, D] -> [KVH, D, w]
        kt_all[:, :, offs[b] * CHUNK : offs[b] * CHUNK + w] = kk.transpose(
            1, 2, 0
        )
        vv = np.zeros((w, KVH, D), dtype=np.float32)
        vv[:Lb] = v_lin[b, :Lb]
        # [cb, 128, KVH, D] -> [KVH, 128, cb, D] -> [KVH, 128, w]
        v_all[:, :, offs[b] * CHUNK : offs[b] * CHUNK + w] = (
            vv.reshape(cb[b], CHUNK, KVH, D)
            .transpose(2, 1, 0, 3)
            .reshape(KVH, CHUNK, w)
        )
    # query [B,Q,H,D] -> [KVH, D, B, Q, G] (t-major, g inner)
    qt_all = (
        query.transpose(2, 3, 0, 1)
        .reshape(KVH, G, D, B, Q)
        .transpose(0, 2, 3, 4, 1)
        .reshape(KVH, D, B * QCOLS)
    )
    qt_all = np.ascontiguousarray(qt_all, dtype=np.float32)
    return [
        {
            "kt": np.ascontiguousarray(kt_all[h]).astype(ml_dtypes.bfloat16),
            "v": np.ascontiguousarray(v_all[h]).astype(ml_dtypes.bfloat16),
            "qt": qt_all[h].astype(ml_dtypes.bfloat16),
        }
        for h in range(KVH)
    ]


def _unpack_outputs(results):
    """Host softmax division + relayout.

    Per core: out_o [B,D,QCOLS] bf16 (unnormalized O^T, q=(t,g) cols) and
    out_l [2,B*QCOLS] f32 where row 0 holds the denominators.
    """
    out = np.empty((B * Q, H * D), dtype=np.float32)
    for h, res in enumerate(results):
        o = np.asarray(res["out_o"], dtype=np.float32)  # [B, D, QCOLS]
        l = np.asarray(res["out_l"], dtype=np.float32)[0].reshape(B, QCOLS)
        o = o / l[:, None, :]
        o = o.reshape(B, D, Q, G).transpose(0, 2, 3, 1).reshape(B * Q, G * D)
        out[:, h * G * D : (h + 1) * G * D] = o
    return out


def kernel(query, k_cache, v_cache, block_tables, seq_lens):
    query = np.asarray(query, dtype=np.float32)
    k_cache = np.asarray(k_cache, dtype=np.float32)
    v_cache = np.asarray(v_cache, dtype=np.float32)
    block_tables = np.asarray(block_tables, dtype=np.int64)
    nc, L, cb, offs = _build(np.asarray(seq_lens))
    in_maps = _pack_inputs(query, k_cache, v_cache, block_tables, L, cb, offs)
    res = run_bass_kernel_spmd(nc, in_maps, core_ids=list(range(N_CORES)))
    return _unpack_outputs(res.results)
